# revision 7
# baseline (speedup 1.0000x reference)
"""Data-parallel linear layer (x @ W.T + bias) on 8 TRN2 NeuronCores.

Shard x over batch: each core computes a (1024 x 2048) @ (2048 x 2048).T
matmul with bf16 inputs (fp32 PSUM accumulate), bias added on DVE, bf16
outputs cast back to fp32 on host.  bf16 halves HBM traffic so the
kernel is cleanly PE-bound: 512 matmuls x 512 cols @ 2.4 GHz ~= 109 us.

Inputs are relaid out on the host so that each input DMA moves 4-16 KiB
per partition line (k-slabs concatenated along the free dim) -- small
per-partition lines (1-2 KiB) cap a DMA queue at ~120-150 GB/s, fat
lines run near the 358 GB/s HBM-per-core limit.

Schedule per core:
 - warmup: 4 matmuls on a memset tile right after the NEFF preamble so
   the PE HAM clock-gate reaches 8/8 (2.4 GHz) shortly after real data
   lands.
 - n=0: k-major (PSUM groups for all 8 m interleave per k) -- compute
   starts as soon as the first x chunk arrives; n=0 drains interleave
   into the last k-step so each PSUM bank frees as its chain completes.
 - n=1..3: m-major (16 k-contiguous matmuls per PSUM group) -- drains
   and output DMAs spread evenly, PE never idles at phase boundaries.
 - the very last group (n=3, m=7) is split into two 256-wide chains so
   the final drain+store is half-size and overlaps the second chain.

DMA rules learned from traces:
 - global emission order MUST match consumption order: the Tile
   scheduler assigns HWDGE completions to 8 sem lanes round-robin in
   emission order with monotonic counters, so a consumer waiting on one
   DMA transitively waits on every earlier-emitted DMA on its lane.
 - sync ring carries x then w[1..3]: ring FIFO order guarantees the
   later-phase weights cannot steal HBM bandwidth from the x stream.
 - scalar ring carries w[0] (consumed in lockstep with x), bias, then
   outputs.
 - w[2]/w[3] tiles recycle w[0]/w[1] buffers (WAR dependency) which
   paces those fetches to consumption.
"""
import numpy as np
import ml_dtypes

import concourse.bass as bass  # noqa: F401
import concourse.mybir as mybir
import concourse.tile as tile
from concourse import bacc, bass_utils

B, IN, OUT = 8192, 2048, 2048
NCORES = 8
BS = B // NCORES      # 1024 batch rows per core
P = 128               # partition dim
NFREE = 512           # one PSUM bank of fp32
KT = IN // P          # 16 contraction tiles
MT = BS // P          # 8 output-row tiles per core
NT = OUT // NFREE     # 4 output-col tiles
WARM_MMS = 4          # bridge PE idle from preamble to first x chunk

XW = KT * BS          # 16384: x relayout free dim
WW = KT * NFREE       # 8192: per-n w relayout free dim

F32 = mybir.dt.float32
BF16 = mybir.dt.bfloat16
NPBF16 = ml_dtypes.bfloat16

TRACE = False
LAST_EXEC_NS = None

_NC_CACHE = {}


def _build():
    if "nc" in _NC_CACHE:
        return _NC_CACHE["nc"]
    nc = bacc.Bacc("TRN2", target_bir_lowering=False, debug=False)
    xT = nc.dram_tensor("xT", [P, XW], BF16, kind="ExternalInput")
    wn = [nc.dram_tensor(f"w{n}", [P, WW], BF16, kind="ExternalInput")
          for n in range(NT)]
    bias_b = nc.dram_tensor("bias_b", [P, OUT], F32, kind="ExternalInput")
    out = nc.dram_tensor("out", [BS, OUT], BF16, kind="ExternalOutput")

    xT_ap = xT.ap()
    wn_ap = [t.ap() for t in wn]
    out_ap = out.ap()

    with tile.TileContext(nc) as tc:
        with tc.tile_pool(name="xp", bufs=1) as xp, \
             tc.tile_pool(name="wp", bufs=2) as wp, \
             tc.tile_pool(name="bp", bufs=1) as bp, \
             tc.tile_pool(name="wu", bufs=2) as wu, \
             tc.tile_pool(name="op", bufs=16) as op, \
             tc.tile_pool(name="pp", bufs=8, space="PSUM") as pp:
            bias_sb = bp.tile([P, OUT], F32, tag="bias", name="bias_sb")
            x_sb = xp.tile([P, XW], BF16, tag="x", name="x_sb")
            w_sb = [None] * NT

            # warmup operands (memset, no DMA dependency)
            wu_s = wu.tile([P, P], BF16, tag="wu", name="wu_s")
            wu_m = wu.tile([P, NFREE], BF16, tag="wu", name="wu_m")
            nc.gpsimd.memset(wu_s[:], 0.0)
            nc.gpsimd.memset(wu_m[:], 0.0)

            def x_slab(k):
                return x_sb[:, k * BS:(k + 1) * BS]

            def w_slab(n, k):
                return w_sb[n][:, k * NFREE:(k + 1) * NFREE]

            def mm(n, k, m, ps_m):
                nc.tensor.matmul(
                    ps_m[:],
                    x_slab(k)[:, m * P:(m + 1) * P],
                    w_slab(n, k)[:],
                    start=(k == 0),
                    stop=(k == KT - 1),
                )

            def drain(n, m, ps_m):
                ot = op.tile([P, NFREE], BF16, tag="o", name=f"o_{n}_{m}")
                nc.vector.tensor_add(
                    ot[:], ps_m[:], bias_sb[:, n * NFREE:(n + 1) * NFREE])
                nc.scalar.dma_start(
                    out_ap[m * P:(m + 1) * P,
                           n * NFREE:(n + 1) * NFREE], ot[:])

            # ---- input DMAs, emitted in consumption order ----
            w_sb[0] = wp.tile([P, WW], BF16, tag="w", name="w_0")
            w_sb[1] = wp.tile([P, WW], BF16, tag="w", name="w_1")
            # first matmul's operands land first: x(k=0,m=0) head, then
            # rest of k=0..1, then w0(k=0), w0(k=1..3)
            nc.sync.dma_start(x_sb[:, :P], xT_ap[:, :P])
            nc.sync.dma_start(x_sb[:, P:2 * BS], xT_ap[:, P:2 * BS])
            nc.scalar.dma_start(w_sb[0][:, :NFREE], wn_ap[0][:, :NFREE])
            nc.scalar.dma_start(
                w_sb[0][:, NFREE:4 * NFREE], wn_ap[0][:, NFREE:4 * NFREE])
            # remaining x in 2-slab chunks (4 KiB lines), w0 in 4-slab
            # chunks (4 KiB lines), interleaved at consumption ratio
            for c in range(1, KT // 2):
                lo, hi = c * 2 * BS, (c + 1) * 2 * BS
                nc.sync.dma_start(x_sb[:, lo:hi], xT_ap[:, lo:hi])
                if c % 2 == 0:
                    wl, wh = c * 2 * NFREE, (c + 2) * 2 * NFREE
                    nc.scalar.dma_start(
                        w_sb[0][:, wl:wh], wn_ap[0][:, wl:wh])
            nc.scalar.dma_start(bias_sb[:], bias_b.ap())
            # w1 behind x on the sync ring; w2/w3 recycle w0/w1 buffers
            for c in range(4):
                lo, hi = c * 4 * NFREE, (c + 1) * 4 * NFREE
                nc.sync.dma_start(w_sb[1][:, lo:hi], wn_ap[1][:, lo:hi])
            w_sb[2] = wp.tile([P, WW], BF16, tag="w", name="w_2")
            w_sb[3] = wp.tile([P, WW], BF16, tag="w", name="w_3")
            for n in (2, 3):
                for c in range(4):
                    lo, hi = c * 4 * NFREE, (c + 1) * 4 * NFREE
                    nc.sync.dma_start(w_sb[n][:, lo:hi], wn_ap[n][:, lo:hi])

            # ---- compute ----
            # n=0: k-major, PSUM groups for all 8 m interleave per k
            ps0 = [pp.tile([P, NFREE], F32, tag="ps", name=f"ps_0_{m}")
                   for m in range(MT)]
            for i in range(WARM_MMS):
                nc.tensor.matmul(ps0[0][:], wu_s[:], wu_m[:],
                                 start=True, stop=True)
            for k in range(KT - 1):
                for m in range(MT):
                    mm(0, k, m, ps0[m])
            # interleave the n=0 drains into the last k-step so each
            # PSUM bank frees right as its chain completes
            for m in range(MT):
                mm(0, KT - 1, m, ps0[m])
                drain(0, m, ps0[m])

            # n=1..3: m-major, k-contiguous accumulation chains
            for n in range(1, NT):
                for m in range(MT):
                    if n == NT - 1 and m == MT - 1:
                        break
                    ps_m = pp.tile([P, NFREE], F32, tag="ps",
                                   name=f"ps_{n}_{m}")
                    for k in range(KT):
                        mm(n, k, m, ps_m)
                    drain(n, m, ps_m)

            # last group (n=3, m=7): two half-width chains so the final
            # drain+store is small and overlaps the second chain
            n, m = NT - 1, MT - 1
            for h in range(2):
                ps_h = pp.tile([P, NFREE // 2], F32, tag="ps",
                               name=f"ps_{n}_{m}_{h}")
                for k in range(KT):
                    nc.tensor.matmul(
                        ps_h[:],
                        x_slab(k)[:, m * P:(m + 1) * P],
                        w_slab(n, k)[:, h * (NFREE // 2):(h + 1) * (NFREE // 2)],
                        start=(k == 0),
                        stop=(k == KT - 1),
                    )
                ot = op.tile([P, NFREE // 2], BF16, tag="o", name=f"o_l{h}")
                noff = n * NFREE + h * (NFREE // 2)
                nc.vector.tensor_add(
                    ot[:], ps_h[:], bias_sb[:, noff:noff + NFREE // 2])
                # final half-stores ride the otherwise-idle SP ring
                nc.sync.dma_start(
                    out_ap[m * P:(m + 1) * P, noff:noff + NFREE // 2], ot[:])
    nc.compile()
    _NC_CACHE["nc"] = nc
    return nc


def kernel(x: np.ndarray, weight: np.ndarray, bias: np.ndarray) -> np.ndarray:
    global LAST_EXEC_NS
    x = np.asarray(x, dtype=np.float32)
    weight = np.asarray(weight, dtype=np.float32)
    bias = np.asarray(bias, dtype=np.float32)

    # relayouts: k-slabs concatenated along the free dim so DMA
    # per-partition lines are 4-16 KiB (see module docstring)
    wT = np.ascontiguousarray(weight.T).astype(NPBF16)   # [IN, OUT]
    w_maps = {}
    for n in range(NT):
        w_n = wT[:, n * NFREE:(n + 1) * NFREE]           # [IN, NFREE]
        w_maps[f"w{n}"] = np.ascontiguousarray(
            w_n.reshape(KT, P, NFREE).transpose(1, 0, 2).reshape(P, WW))
    bias_b = np.ascontiguousarray(
        np.broadcast_to(bias[None, :], (P, OUT)), dtype=np.float32)

    xT = np.ascontiguousarray(x.T).astype(NPBF16)        # [IN, B]
    in_maps = []
    for c in range(NCORES):
        xc = xT[:, c * BS:(c + 1) * BS]                  # [IN, BS]
        x2 = np.ascontiguousarray(
            xc.reshape(KT, P, BS).transpose(1, 0, 2).reshape(P, XW))
        in_maps.append({"xT": x2, "bias_b": bias_b, **w_maps})

    nc = _build()
    res = bass_utils.run_bass_kernel_spmd(
        nc, in_maps, core_ids=list(range(NCORES)), trace=TRACE)
    LAST_EXEC_NS = res.exec_time_ns

    return np.concatenate(
        [r["out"].astype(np.float32) for r in res.results], axis=0)


# revision 8
# speedup vs baseline: 1.0001x; 1.0001x over previous
"""Data-parallel linear layer (x @ W.T + bias) on 8 TRN2 NeuronCores.

Shard x over batch: each core computes a (1024 x 2048) @ (2048 x 2048).T
matmul with bf16 inputs (fp32 PSUM accumulate), bias added on DVE, bf16
outputs cast back to fp32 on host.  bf16 halves HBM traffic so the
kernel is cleanly PE-bound: 512 matmuls x 512 cols @ 2.4 GHz ~= 109 us.

Inputs are relaid out on the host so that each input DMA moves 4-16 KiB
per partition line (k-slabs concatenated along the free dim) -- small
per-partition lines (1-2 KiB) cap a DMA queue at ~120-150 GB/s, fat
lines run near the 358 GB/s HBM-per-core limit.

Schedule per core:
 - warmup: 4 matmuls on a memset tile right after the NEFF preamble so
   the PE HAM clock-gate reaches 8/8 (2.4 GHz) shortly after real data
   lands.
 - n=0: k-major (PSUM groups for all 8 m interleave per k) -- compute
   starts as soon as the first x chunk arrives; n=0 drains interleave
   into the last k-step so each PSUM bank frees as its chain completes.
 - n=1..3: m-major (16 k-contiguous matmuls per PSUM group) -- drains
   and output DMAs spread evenly, PE never idles at phase boundaries.
 - the very last group (n=3, m=7) is split into two 256-wide chains so
   the final drain+store is half-size and overlaps the second chain.

DMA rules learned from traces:
 - global emission order MUST match consumption order: the Tile
   scheduler assigns HWDGE completions to 8 sem lanes round-robin in
   emission order with monotonic counters, so a consumer waiting on one
   DMA transitively waits on every earlier-emitted DMA on its lane.
 - sync ring carries x then w[1..3]: ring FIFO order guarantees the
   later-phase weights cannot steal HBM bandwidth from the x stream.
 - scalar ring carries w[0] (consumed in lockstep with x), bias, then
   outputs.
 - w[2]/w[3] tiles recycle w[0]/w[1] buffers (WAR dependency) which
   paces those fetches to consumption.
"""
import numpy as np
import ml_dtypes

import concourse.bass as bass  # noqa: F401
import concourse.mybir as mybir
import concourse.tile as tile
from concourse import bacc, bass_utils

B, IN, OUT = 8192, 2048, 2048
NCORES = 8
BS = B // NCORES      # 1024 batch rows per core
P = 128               # partition dim
NFREE = 512           # one PSUM bank of fp32
KT = IN // P          # 16 contraction tiles
MT = BS // P          # 8 output-row tiles per core
NT = OUT // NFREE     # 4 output-col tiles
WARM_MMS = 6          # bridge PE idle from preamble to first x chunk

XW = KT * BS          # 16384: x relayout free dim
WW = KT * NFREE       # 8192: per-n w relayout free dim

F32 = mybir.dt.float32
BF16 = mybir.dt.bfloat16
NPBF16 = ml_dtypes.bfloat16

TRACE = False
LAST_EXEC_NS = None

_NC_CACHE = {}


def _build():
    if "nc" in _NC_CACHE:
        return _NC_CACHE["nc"]
    nc = bacc.Bacc("TRN2", target_bir_lowering=False, debug=False)
    xT = nc.dram_tensor("xT", [P, XW], BF16, kind="ExternalInput")
    wn = [nc.dram_tensor(f"w{n}", [P, WW], BF16, kind="ExternalInput")
          for n in range(NT)]
    bias_b = nc.dram_tensor("bias_b", [P, OUT], F32, kind="ExternalInput")
    out = nc.dram_tensor("out", [BS, OUT], BF16, kind="ExternalOutput")

    xT_ap = xT.ap()
    wn_ap = [t.ap() for t in wn]
    out_ap = out.ap()

    with tile.TileContext(nc) as tc:
        with tc.tile_pool(name="xp", bufs=1) as xp, \
             tc.tile_pool(name="wp", bufs=4) as wp, \
             tc.tile_pool(name="bp", bufs=1) as bp, \
             tc.tile_pool(name="wu", bufs=2) as wu, \
             tc.tile_pool(name="op", bufs=16) as op, \
             tc.tile_pool(name="pp", bufs=8, space="PSUM") as pp:
            bias_sb = bp.tile([P, OUT], F32, tag="bias", name="bias_sb")
            x_sb = xp.tile([P, XW], BF16, tag="x", name="x_sb")
            w_sb = [None] * NT

            # warmup operands (memset, no DMA dependency)
            wu_s = wu.tile([P, P], BF16, tag="wu", name="wu_s")
            wu_m = wu.tile([P, NFREE], BF16, tag="wu", name="wu_m")
            nc.gpsimd.memset(wu_s[:], 0.0)
            nc.gpsimd.memset(wu_m[:], 0.0)

            def x_slab(k):
                return x_sb[:, k * BS:(k + 1) * BS]

            def w_slab(n, k):
                return w_sb[n][:, k * NFREE:(k + 1) * NFREE]

            def mm(n, k, m, ps_m):
                nc.tensor.matmul(
                    ps_m[:],
                    x_slab(k)[:, m * P:(m + 1) * P],
                    w_slab(n, k)[:],
                    start=(k == 0),
                    stop=(k == KT - 1),
                )

            def drain(n, m, ps_m):
                ot = op.tile([P, NFREE], BF16, tag="o", name=f"o_{n}_{m}")
                nc.vector.tensor_add(
                    ot[:], ps_m[:], bias_sb[:, n * NFREE:(n + 1) * NFREE])
                nc.scalar.dma_start(
                    out_ap[m * P:(m + 1) * P,
                           n * NFREE:(n + 1) * NFREE], ot[:])

            # ---- input DMAs, emitted in consumption order ----
            w_sb[0] = wp.tile([P, WW], BF16, tag="w", name="w_0")
            w_sb[1] = wp.tile([P, WW], BF16, tag="w", name="w_1")
            # ramped chunk sizes: tiny first tiles (each DMA's completion
            # sem fires ~1.5-2us after the last byte, so the first
            # operands must ride minimal transfers), fat tails for rate
            nc.sync.dma_start(x_sb[:, :P], xT_ap[:, :P])
            nc.sync.dma_start(x_sb[:, P:BS], xT_ap[:, P:BS])
            nc.scalar.dma_start(w_sb[0][:, :NFREE], wn_ap[0][:, :NFREE])
            nc.sync.dma_start(x_sb[:, BS:2 * BS], xT_ap[:, BS:2 * BS])
            nc.scalar.dma_start(
                w_sb[0][:, NFREE:2 * NFREE], wn_ap[0][:, NFREE:2 * NFREE])
            for c in ((2, 4), (4, 8), (8, 12), (12, 16)):
                lo, hi = c[0] * BS, c[1] * BS
                nc.sync.dma_start(x_sb[:, lo:hi], xT_ap[:, lo:hi])
                wl, wh = c[0] * NFREE, c[1] * NFREE
                nc.scalar.dma_start(w_sb[0][:, wl:wh], wn_ap[0][:, wl:wh])
            nc.scalar.dma_start(bias_sb[:], bias_b.ap())
            # w1..w3 behind x on the sync ring (FIFO paces them); two
            # fat DMAs each so their consumers see few sem waits
            w_sb[2] = wp.tile([P, WW], BF16, tag="w", name="w_2")
            w_sb[3] = wp.tile([P, WW], BF16, tag="w", name="w_3")
            for n in (1, 2, 3):
                h = WW // 2
                nc.sync.dma_start(w_sb[n][:, :h], wn_ap[n][:, :h])
                nc.sync.dma_start(w_sb[n][:, h:], wn_ap[n][:, h:])

            # ---- compute ----
            # n=0: k-major, PSUM groups for all 8 m interleave per k
            ps0 = [pp.tile([P, NFREE], F32, tag="ps", name=f"ps_0_{m}")
                   for m in range(MT)]
            for i in range(WARM_MMS):
                nc.tensor.matmul(ps0[0][:], wu_s[:], wu_m[:],
                                 start=True, stop=True)
            for k in range(KT - 1):
                for m in range(MT):
                    mm(0, k, m, ps0[m])
            # interleave the n=0 drains into the last k-step so each
            # PSUM bank frees right as its chain completes
            for m in range(MT):
                mm(0, KT - 1, m, ps0[m])
                drain(0, m, ps0[m])

            # n=1..3: m-major, k-contiguous accumulation chains
            for n in range(1, NT):
                for m in range(MT):
                    if n == NT - 1 and m == MT - 1:
                        break
                    ps_m = pp.tile([P, NFREE], F32, tag="ps",
                                   name=f"ps_{n}_{m}")
                    for k in range(KT):
                        mm(n, k, m, ps_m)
                    drain(n, m, ps_m)

            # last group (n=3, m=7): two half-width chains so the final
            # drain+store is small and overlaps the second chain
            n, m = NT - 1, MT - 1
            for h in range(2):
                ps_h = pp.tile([P, NFREE // 2], F32, tag="ps",
                               name=f"ps_{n}_{m}_{h}")
                for k in range(KT):
                    nc.tensor.matmul(
                        ps_h[:],
                        x_slab(k)[:, m * P:(m + 1) * P],
                        w_slab(n, k)[:, h * (NFREE // 2):(h + 1) * (NFREE // 2)],
                        start=(k == 0),
                        stop=(k == KT - 1),
                    )
                ot = op.tile([P, NFREE // 2], BF16, tag="o", name=f"o_l{h}")
                noff = n * NFREE + h * (NFREE // 2)
                nc.vector.tensor_add(
                    ot[:], ps_h[:], bias_sb[:, noff:noff + NFREE // 2])
                # final half-stores ride the otherwise-idle SP ring
                nc.sync.dma_start(
                    out_ap[m * P:(m + 1) * P, noff:noff + NFREE // 2], ot[:])
    nc.compile()
    _NC_CACHE["nc"] = nc
    return nc


def kernel(x: np.ndarray, weight: np.ndarray, bias: np.ndarray) -> np.ndarray:
    global LAST_EXEC_NS
    x = np.asarray(x, dtype=np.float32)
    weight = np.asarray(weight, dtype=np.float32)
    bias = np.asarray(bias, dtype=np.float32)

    # relayouts: k-slabs concatenated along the free dim so DMA
    # per-partition lines are 4-16 KiB (see module docstring)
    wT = np.ascontiguousarray(weight.T).astype(NPBF16)   # [IN, OUT]
    w_maps = {}
    for n in range(NT):
        w_n = wT[:, n * NFREE:(n + 1) * NFREE]           # [IN, NFREE]
        w_maps[f"w{n}"] = np.ascontiguousarray(
            w_n.reshape(KT, P, NFREE).transpose(1, 0, 2).reshape(P, WW))
    bias_b = np.ascontiguousarray(
        np.broadcast_to(bias[None, :], (P, OUT)), dtype=np.float32)

    xT = np.ascontiguousarray(x.T).astype(NPBF16)        # [IN, B]
    in_maps = []
    for c in range(NCORES):
        xc = xT[:, c * BS:(c + 1) * BS]                  # [IN, BS]
        x2 = np.ascontiguousarray(
            xc.reshape(KT, P, BS).transpose(1, 0, 2).reshape(P, XW))
        in_maps.append({"xT": x2, "bias_b": bias_b, **w_maps})

    nc = _build()
    res = bass_utils.run_bass_kernel_spmd(
        nc, in_maps, core_ids=list(range(NCORES)), trace=TRACE)
    LAST_EXEC_NS = res.exec_time_ns

    return np.concatenate(
        [r["out"].astype(np.float32) for r in res.results], axis=0)


# revision 9
# speedup vs baseline: 1.0140x; 1.0139x over previous
"""Data-parallel linear layer (x @ W.T + bias) on 8 TRN2 NeuronCores.

Shard x over batch: each core computes a (1024 x 2048) @ (2048 x 2048).T
matmul with bf16 inputs (fp32 PSUM accumulate), bias added on DVE, bf16
outputs cast back to fp32 on host.  bf16 halves HBM traffic so the
kernel is cleanly PE-bound: 512 matmuls x 512 cols @ 2.4 GHz ~= 109 us.

Inputs are relaid out on the host so that each input DMA moves 4-16 KiB
per partition line (k-slabs concatenated along the free dim) -- small
per-partition lines (1-2 KiB) cap a DMA queue at ~120-150 GB/s, fat
lines run near the 358 GB/s HBM-per-core limit.

Schedule per core:
 - warmup: 4 matmuls on a memset tile right after the NEFF preamble so
   the PE HAM clock-gate reaches 8/8 (2.4 GHz) shortly after real data
   lands.
 - n=0: k-major (PSUM groups for all 8 m interleave per k) -- compute
   starts as soon as the first x chunk arrives; n=0 drains interleave
   into the last k-step so each PSUM bank frees as its chain completes.
 - n=1..3: m-major (16 k-contiguous matmuls per PSUM group) -- drains
   and output DMAs spread evenly, PE never idles at phase boundaries.
 - the very last group (n=3, m=7) is split into two 256-wide chains so
   the final drain+store is half-size and overlaps the second chain.

DMA rules learned from traces:
 - global emission order MUST match consumption order: the Tile
   scheduler assigns HWDGE completions to 8 sem lanes round-robin in
   emission order with monotonic counters, so a consumer waiting on one
   DMA transitively waits on every earlier-emitted DMA on its lane.
 - sync ring carries x then w[1..3]: ring FIFO order guarantees the
   later-phase weights cannot steal HBM bandwidth from the x stream.
 - scalar ring carries w[0] (consumed in lockstep with x), bias, then
   outputs.
 - w[2]/w[3] tiles recycle w[0]/w[1] buffers (WAR dependency) which
   paces those fetches to consumption.
"""
import numpy as np
import ml_dtypes

import concourse.bass as bass  # noqa: F401
import concourse.mybir as mybir
import concourse.tile as tile
from concourse import bacc, bass_utils

B, IN, OUT = 8192, 2048, 2048
NCORES = 8
BS = B // NCORES      # 1024 batch rows per core
P = 128               # partition dim
NFREE = 512           # one PSUM bank of fp32
KT = IN // P          # 16 contraction tiles
MT = BS // P          # 8 output-row tiles per core
NT = OUT // NFREE     # 4 output-col tiles
WARM_MMS = 6          # bridge PE idle from preamble to first x chunk

XW = KT * BS          # 16384: x relayout free dim
WW = KT * NFREE       # 8192: per-n w relayout free dim

F32 = mybir.dt.float32
BF16 = mybir.dt.bfloat16
NPBF16 = ml_dtypes.bfloat16

TRACE = False
LAST_EXEC_NS = None

_NC_CACHE = {}


def _build():
    if "nc" in _NC_CACHE:
        return _NC_CACHE["nc"]
    nc = bacc.Bacc("TRN2", target_bir_lowering=False, debug=False)
    xT = nc.dram_tensor("xT", [P, XW], BF16, kind="ExternalInput")
    wn = [nc.dram_tensor(f"w{n}", [P, WW], BF16, kind="ExternalInput")
          for n in range(NT)]
    bias_b = nc.dram_tensor("bias_b", [P, OUT], F32, kind="ExternalInput")
    out = nc.dram_tensor("out", [BS, OUT], BF16, kind="ExternalOutput")

    xT_ap = xT.ap()
    wn_ap = [t.ap() for t in wn]
    out_ap = out.ap()

    with tile.TileContext(nc) as tc:
        with tc.tile_pool(name="xp", bufs=1) as xp, \
             tc.tile_pool(name="wp", bufs=4) as wp, \
             tc.tile_pool(name="bp", bufs=1) as bp, \
             tc.tile_pool(name="wu", bufs=2) as wu, \
             tc.tile_pool(name="op", bufs=16) as op, \
             tc.tile_pool(name="pp", bufs=8, space="PSUM") as pp:
            bias_sb = bp.tile([P, OUT], F32, tag="bias", name="bias_sb")
            x_sb = xp.tile([P, XW], BF16, tag="x", name="x_sb")
            w_sb = [None] * NT

            # warmup operands (memset, no DMA dependency)
            wu_s = wu.tile([P, P], BF16, tag="wu", name="wu_s")
            wu_m = wu.tile([P, NFREE], BF16, tag="wu", name="wu_m")
            nc.gpsimd.memset(wu_s[:], 0.0)
            nc.gpsimd.memset(wu_m[:], 0.0)

            def x_slab(k):
                return x_sb[:, k * BS:(k + 1) * BS]

            def w_slab(n, k):
                return w_sb[n][:, k * NFREE:(k + 1) * NFREE]

            def mm(n, k, m, ps_m):
                nc.tensor.matmul(
                    ps_m[:],
                    x_slab(k)[:, m * P:(m + 1) * P],
                    w_slab(n, k)[:],
                    start=(k == 0),
                    stop=(k == KT - 1),
                )

            def drain(n, m, ps_m):
                ot = op.tile([P, NFREE], BF16, tag="o", name=f"o_{n}_{m}")
                nc.vector.tensor_add(
                    ot[:], ps_m[:], bias_sb[:, n * NFREE:(n + 1) * NFREE])
                nc.scalar.dma_start(
                    out_ap[m * P:(m + 1) * P,
                           n * NFREE:(n + 1) * NFREE], ot[:])

            # ---- input DMAs, emitted in consumption order ----
            w_sb[0] = wp.tile([P, WW], BF16, tag="w", name="w_0")
            w_sb[1] = wp.tile([P, WW], BF16, tag="w", name="w_1")
            # ramped chunk sizes: tiny first tiles (each DMA's
            # completion sem fires ~1.5-2us after the last byte, so the
            # first operands must ride minimal transfers) and per-slab
            # granularity for k<4 (the k-major loop consumes a slab per
            # 1.7us; coarse chunks would stall it on whole-chunk
            # receipts), then 2-slab chunks for pipelined depth
            nc.sync.dma_start(x_sb[:, :P], xT_ap[:, :P])
            nc.sync.dma_start(x_sb[:, P:BS], xT_ap[:, P:BS])
            nc.scalar.dma_start(w_sb[0][:, :NFREE], wn_ap[0][:, :NFREE])
            for k in (1, 2, 3):
                nc.sync.dma_start(
                    x_sb[:, k * BS:(k + 1) * BS],
                    xT_ap[:, k * BS:(k + 1) * BS])
                nc.scalar.dma_start(
                    w_sb[0][:, k * NFREE:(k + 1) * NFREE],
                    wn_ap[0][:, k * NFREE:(k + 1) * NFREE])
            for c in ((4, 6), (6, 8), (8, 10), (10, 12), (12, 14),
                      (14, 16)):
                lo, hi = c[0] * BS, c[1] * BS
                nc.sync.dma_start(x_sb[:, lo:hi], xT_ap[:, lo:hi])
                if c[0] % 4 == 0:
                    wl, wh = c[0] * NFREE, (c[0] + 4) * NFREE
                    nc.scalar.dma_start(
                        w_sb[0][:, wl:wh], wn_ap[0][:, wl:wh])
            nc.scalar.dma_start(bias_sb[:], bias_b.ap())
            # w1..w3 behind x on the sync ring (FIFO paces them); two
            # fat DMAs each so their consumers see few sem waits
            w_sb[2] = wp.tile([P, WW], BF16, tag="w", name="w_2")
            w_sb[3] = wp.tile([P, WW], BF16, tag="w", name="w_3")
            for n in (1, 2, 3):
                h = WW // 2
                nc.sync.dma_start(w_sb[n][:, :h], wn_ap[n][:, :h])
                nc.sync.dma_start(w_sb[n][:, h:], wn_ap[n][:, h:])

            # ---- compute ----
            # n=0: k-major, PSUM groups for all 8 m interleave per k
            ps0 = [pp.tile([P, NFREE], F32, tag="ps", name=f"ps_0_{m}")
                   for m in range(MT)]
            for i in range(WARM_MMS):
                nc.tensor.matmul(ps0[0][:], wu_s[:], wu_m[:],
                                 start=True, stop=True)
            for k in range(KT - 1):
                for m in range(MT):
                    mm(0, k, m, ps0[m])
            # interleave the n=0 drains into the last k-step so each
            # PSUM bank frees right as its chain completes
            for m in range(MT):
                mm(0, KT - 1, m, ps0[m])
                drain(0, m, ps0[m])

            # n=1..3: m-major, k-contiguous accumulation chains
            for n in range(1, NT):
                for m in range(MT):
                    if n == NT - 1 and m == MT - 1:
                        break
                    ps_m = pp.tile([P, NFREE], F32, tag="ps",
                                   name=f"ps_{n}_{m}")
                    for k in range(KT):
                        mm(n, k, m, ps_m)
                    drain(n, m, ps_m)

            # last group (n=3, m=7): two half-width chains so the final
            # drain+store is small and overlaps the second chain
            n, m = NT - 1, MT - 1
            for h in range(2):
                ps_h = pp.tile([P, NFREE // 2], F32, tag="ps",
                               name=f"ps_{n}_{m}_{h}")
                for k in range(KT):
                    nc.tensor.matmul(
                        ps_h[:],
                        x_slab(k)[:, m * P:(m + 1) * P],
                        w_slab(n, k)[:, h * (NFREE // 2):(h + 1) * (NFREE // 2)],
                        start=(k == 0),
                        stop=(k == KT - 1),
                    )
                ot = op.tile([P, NFREE // 2], BF16, tag="o", name=f"o_l{h}")
                noff = n * NFREE + h * (NFREE // 2)
                nc.vector.tensor_add(
                    ot[:], ps_h[:], bias_sb[:, noff:noff + NFREE // 2])
                # final half-stores ride the otherwise-idle SP ring
                nc.sync.dma_start(
                    out_ap[m * P:(m + 1) * P, noff:noff + NFREE // 2], ot[:])
    nc.compile()
    _NC_CACHE["nc"] = nc
    return nc


def kernel(x: np.ndarray, weight: np.ndarray, bias: np.ndarray) -> np.ndarray:
    global LAST_EXEC_NS
    x = np.asarray(x, dtype=np.float32)
    weight = np.asarray(weight, dtype=np.float32)
    bias = np.asarray(bias, dtype=np.float32)

    # relayouts: k-slabs concatenated along the free dim so DMA
    # per-partition lines are 4-16 KiB (see module docstring)
    wT = np.ascontiguousarray(weight.T).astype(NPBF16)   # [IN, OUT]
    w_maps = {}
    for n in range(NT):
        w_n = wT[:, n * NFREE:(n + 1) * NFREE]           # [IN, NFREE]
        w_maps[f"w{n}"] = np.ascontiguousarray(
            w_n.reshape(KT, P, NFREE).transpose(1, 0, 2).reshape(P, WW))
    bias_b = np.ascontiguousarray(
        np.broadcast_to(bias[None, :], (P, OUT)), dtype=np.float32)

    xT = np.ascontiguousarray(x.T).astype(NPBF16)        # [IN, B]
    in_maps = []
    for c in range(NCORES):
        xc = xT[:, c * BS:(c + 1) * BS]                  # [IN, BS]
        x2 = np.ascontiguousarray(
            xc.reshape(KT, P, BS).transpose(1, 0, 2).reshape(P, XW))
        in_maps.append({"xT": x2, "bias_b": bias_b, **w_maps})

    nc = _build()
    res = bass_utils.run_bass_kernel_spmd(
        nc, in_maps, core_ids=list(range(NCORES)), trace=TRACE)
    LAST_EXEC_NS = res.exec_time_ns

    return np.concatenate(
        [r["out"].astype(np.float32) for r in res.results], axis=0)
